# revision 7
# baseline (speedup 1.0000x reference)
"""MoE layer (dense experts) on 8 Trainium2 NeuronCores via Bass/Tile.

Problem (hardcoded shapes):
  x        [4, 2048, 1024] f32
  gate_w   [1024, 8] f32, gate_b [8] f32
  expert_w [8, 1024, 1024] f32, expert_b [8, 1024] f32
  out[b,t,p] = sum_e softmax(x @ gate_w + gate_b)[b,t,e]
               * (x @ expert_w[e] + expert_b[e])[b,t,p]

Sharding: data-parallel over tokens. 8192 tokens are split into 8 shards of
1024; every core gets the full gate/expert weights (replicated) and computes
its token shard end-to-end. No collectives.

Per-core kernel (x pre-transposed on host so the contraction dim is the
partition dim for both matmul operands):
  - gating logits per token tile via PE matmuls accumulated over d-tiles
    (gate_b broadcast in via a K=1 ones x gate_b rank-1 matmul), softmax on
    DVE/ACT, normalized gates also transposed on PE for the bias-mix matmul
  - expert e: psum[t128, p512] accumulates sum_d xT[d,t].T @ w_e[d,p] over
    8 d-tiles; d is the outer loop within a 4-token-tile half so compute
    starts as soon as the first w d-tile DMA lands
  - gate-weighted sum on DVE: acc = psum_e * g[:,e] + acc (one fused
    scalar_tensor_tensor per psum tile)
  - expert_b handled once per output tile: psum_b = gT.T @ expert_b (K=8
    matmul, gate-weighted bias mix), final out = acc + psum_b
Matmul dtype: bf16 (default) or float32r (full-rate fp32 streaming, ~1.2x
slower, ~16x more accurate) via MOE_MM_DTYPE in {bf16, fp32r, fp32}.
"""

import os
from contextlib import ExitStack

import numpy as np

import concourse.bacc as bacc
import concourse.bass as bass
import concourse.mybir as mybir
import concourse.tile as tile
from concourse.bass_utils import run_bass_kernel_spmd

B, T, D, E, P = 4, 2048, 1024, 8, 1024
N_CORES = 8
TOK = B * T                # 8192 tokens
TS = TOK // N_CORES        # 1024 tokens per core
DT = D // 128              # 8 contraction tiles
TT = TS // 128             # 8 token tiles per core
PCHUNK = 512               # psum bank free size (f32)
PC = P // PCHUNK           # 2 p-chunks
TH = 4                     # token tiles per half (TH*PC = 8 psum banks)

_F32 = mybir.dt.float32
_BF16 = mybir.dt.bfloat16

MM_DTYPE = os.environ.get("MOE_MM_DTYPE", "bf16")
TRACE = os.environ.get("MOE_TRACE", "0") == "1"

_mm_dt = {
    "fp32r": mybir.dt.float32r,
    "bf16": mybir.dt.bfloat16,
    "fp32": mybir.dt.float32,
}

_build_cache = {}


def _build(mode: str) -> bass.Bass:
    mm = _mm_dt[mode]
    nc = bacc.Bacc("TRN2", target_bir_lowering=False, debug=False,
                   num_devices=N_CORES)

    xT = nc.dram_tensor("xT", [D, TS], mm, kind="ExternalInput").ap()
    gw = nc.dram_tensor("gate_w", [D, E], mm, kind="ExternalInput").ap()
    gb = nc.dram_tensor("gate_b", [1, E], mm, kind="ExternalInput").ap()
    ew = nc.dram_tensor("expert_w", [E, D, P], mm, kind="ExternalInput").ap()
    eb = nc.dram_tensor("expert_b", [E, P], _BF16, kind="ExternalInput").ap()
    ones = nc.dram_tensor("ones", [1, 128], mm, kind="ExternalInput").ap()
    ident = nc.dram_tensor("ident", [128, 128], _F32, kind="ExternalInput").ap()
    out = nc.dram_tensor("out", [TS, P], _F32, kind="ExternalOutput").ap()

    out_t = out.rearrange("(tt tp) p -> tp tt p", tp=128)
    xT_t = xT.rearrange("(dt dp) t -> dp dt t", dp=128)

    with tile.TileContext(nc) as tc, ExitStack() as ctx:
        consts = ctx.enter_context(tc.tile_pool(name="consts", bufs=1))
        w_pool = ctx.enter_context(tc.tile_pool(name="w", bufs=12))
        stage_pool = ctx.enter_context(tc.tile_pool(name="stage", bufs=4))
        stats = ctx.enter_context(tc.tile_pool(name="stats", bufs=4))
        psum = ctx.enter_context(tc.tile_pool(name="psum", bufs=8, space="PSUM"))

        # Small resident inputs first, then xT and expert-0 weights
        # interleaved per d-tile so the expert-0 pipeline fills ASAP.
        ones_sb = consts.tile([1, 128], mm, name="ones_sb")
        nc.sync.dma_start(ones_sb[:, :], ones)
        gb_sb = consts.tile([1, E], mm, name="gb_sb")
        nc.sync.dma_start(gb_sb[:, :], gb)
        gw_sb = consts.tile([128, DT, E], mm, name="gw_sb")
        nc.sync.dma_start(gw_sb[:, :, :], gw.rearrange("(dt dp) e -> dp dt e", dp=128))
        eb_sb = consts.tile([E, P], _BF16, name="eb_sb")
        nc.sync.dma_start(eb_sb[:, :], eb)
        id_sb = consts.tile([128, 128], _F32, name="id_sb")
        nc.sync.dma_start(id_sb[:, :], ident)

        xt = consts.tile([128, DT, TS], mm, name="xt")
        w0 = []
        for di in range(DT):
            nc.sync.dma_start(xt[:, di, :], xT_t[:, di, :])
            w_tile = w_pool.tile([128, P], mm, name=f"wt0_{di}", tag="wt")
            nc.sync.dma_start(w_tile[:, :], ew[0, di * 128:(di + 1) * 128, :])
            w0.append(w_tile)

        g_sb = consts.tile([128, TT, E], _F32, name="g_sb")
        gt_sb = consts.tile([E, TS], _BF16, name="gt_sb")
        acc = consts.tile([128, TT, P], _F32, name="acc")

        # --- gating: g = softmax(x @ gate_w + gate_b), plus gT for the
        # bias-mix matmul ---
        for ti in range(TT):
            ps_g = psum.tile([128, PCHUNK], _F32, name="ps_g", tag="ps")
            lg = ps_g[:, :E]
            nc.tensor.matmul(lg, ones_sb[:1, :], gb_sb[:1, :],
                             start=True, stop=False)
            for di in range(DT):
                nc.tensor.matmul(lg, xt[:, di, ti * 128:(ti + 1) * 128],
                                 gw_sb[:, di, :],
                                 start=False, stop=(di == DT - 1))
            negmax = stats.tile([128, 1], _F32, name="negmax")
            nc.vector.tensor_reduce(negmax[:, :], lg, axis=mybir.AxisListType.X,
                                    op=mybir.AluOpType.max, negate=True)
            gexp = g_sb[:, ti, :]
            esum = stats.tile([128, 1], _F32, name="esum")
            nc.scalar.activation(gexp, lg, mybir.ActivationFunctionType.Exp,
                                 bias=negmax[:, :], scale=1.0,
                                 accum_out=esum[:, :])
            rec = stats.tile([128, 1], _F32, name="rec")
            nc.vector.reciprocal(rec[:, :], esum[:, :])
            nc.vector.tensor_scalar_mul(gexp, gexp, rec[:, :])
            # gT[e, t] for the expert_b bias-mix matmul
            ps_t = psum.tile([128, PCHUNK], _F32, name="ps_t", tag="ps")
            gt_ps = ps_t[:E, :128]
            nc.tensor.transpose(gt_ps, gexp, id_sb[:, :])
            nc.scalar.copy(gt_sb[:, ti * 128:(ti + 1) * 128], gt_ps)

        # --- experts ---
        def epilogue(e, ti, pc, ps):
            g_col = g_sb[:, ti, e:e + 1]
            acc_sl = acc[:, ti, pc * PCHUNK:(pc + 1) * PCHUNK]
            if e == 0:
                nc.vector.tensor_scalar_mul(acc_sl, ps[:, :], g_col)
            else:
                nc.vector.scalar_tensor_tensor(
                    acc_sl, ps[:, :], g_col, acc_sl,
                    op0=mybir.AluOpType.mult, op1=mybir.AluOpType.add)
            if e == E - 1:
                # gate-weighted expert_b mix + final store
                ps_b = psum.tile([128, PCHUNK], _F32,
                                 name=f"psb{ti}_{pc}", tag="ps")
                nc.tensor.matmul(
                    ps_b[:, :], gt_sb[:, ti * 128:(ti + 1) * 128],
                    eb_sb[:, pc * PCHUNK:(pc + 1) * PCHUNK],
                    start=True, stop=True)
                stg = stage_pool.tile([128, PCHUNK], _F32, name="stg")
                nc.vector.tensor_add(stg[:, :], acc_sl, ps_b[:, :])
                nc.sync.dma_start(
                    out_t[:, ti, pc * PCHUNK:(pc + 1) * PCHUNK], stg[:, :])

        for e in range(E):
            if e == 0:
                wt = w0
            else:
                wt = []
                for di in range(DT):
                    w_tile = w_pool.tile([128, P], mm, name=f"wt{e}_{di}",
                                         tag="wt")
                    nc.sync.dma_start(w_tile[:, :],
                                      ew[e, di * 128:(di + 1) * 128, :])
                    wt.append(w_tile)
            if e == 0:
                # d-outer: start computing as soon as the first xT/w0
                # d-tiles land (DMA-bound ramp-in phase).
                for half in range(TT // TH):
                    tis = range(half * TH, (half + 1) * TH)
                    ps_grp = {}
                    for ti in tis:
                        for pc in range(PC):
                            ps_grp[ti, pc] = psum.tile(
                                [128, PCHUNK], _F32,
                                name=f"ps{e}_{ti}_{pc}", tag="ps")
                    for di in range(DT):
                        for ti in tis:
                            for pc in range(PC):
                                nc.tensor.matmul(
                                    ps_grp[ti, pc][:, :],
                                    xt[:, di, ti * 128:(ti + 1) * 128],
                                    wt[di][:, pc * PCHUNK:(pc + 1) * PCHUNK],
                                    start=(di == 0), stop=(di == DT - 1))
                    for ti in tis:
                        for pc in range(PC):
                            epilogue(e, ti, pc, ps_grp[ti, pc])
            else:
                # group-major: each output tile finishes its d-loop early so
                # the DVE epilogue chain spreads across the expert phase.
                for ti in range(TT):
                    for pc in range(PC):
                        ps = psum.tile([128, PCHUNK], _F32,
                                       name=f"ps{e}_{ti}_{pc}", tag="ps")
                        for di in range(DT):
                            nc.tensor.matmul(
                                ps[:, :], xt[:, di, ti * 128:(ti + 1) * 128],
                                wt[di][:, pc * PCHUNK:(pc + 1) * PCHUNK],
                                start=(di == 0), stop=(di == DT - 1))
                        epilogue(e, ti, pc, ps)

    nc.compile()
    return nc


def _get_module(mode: str) -> bass.Bass:
    if mode not in _build_cache:
        _build_cache[mode] = _build(mode)
    return _build_cache[mode]


_last_results = None


def _host_inputs(x, gate_w, gate_b, expert_w, expert_b, mode):
    import ml_dtypes
    np_dt = ml_dtypes.bfloat16 if mode == "bf16" else np.float32

    x_flat = np.asarray(x, dtype=np.float32).reshape(TOK, D)
    gw_h = np.ascontiguousarray(np.asarray(gate_w, np.float32)).astype(np_dt)
    gb_h = np.asarray(gate_b, np.float32).reshape(1, E).astype(np_dt)
    ew_h = np.ascontiguousarray(np.asarray(expert_w, np.float32)).astype(np_dt)
    eb_h = np.asarray(expert_b, np.float32).astype(ml_dtypes.bfloat16)
    ones_h = np.ones((1, 128), dtype=np_dt)
    ident_h = np.eye(128, dtype=np.float32)

    in_maps = []
    for c in range(N_CORES):
        shard = x_flat[c * TS:(c + 1) * TS]                  # [TS, D]
        xT_h = np.ascontiguousarray(shard.T).astype(np_dt)   # [D, TS]
        in_maps.append({
            "xT": xT_h, "gate_w": gw_h, "gate_b": gb_h,
            "expert_w": ew_h, "expert_b": eb_h, "ones": ones_h,
            "ident": ident_h,
        })
    return in_maps


def kernel(x, gate_w, gate_b, expert_w, expert_b):
    global _last_results
    mode = MM_DTYPE
    nc = _get_module(mode)
    in_maps = _host_inputs(x, gate_w, gate_b, expert_w, expert_b, mode)

    res = run_bass_kernel_spmd(nc, in_maps, core_ids=list(range(N_CORES)),
                               trace=TRACE)
    _last_results = res

    out = np.concatenate([res.results[c]["out"] for c in range(N_CORES)], axis=0)
    return out.reshape(B, T, P).astype(np.float32)


# revision 8
# speedup vs baseline: 1.0252x; 1.0252x over previous
"""MoE layer (dense experts) on 8 Trainium2 NeuronCores via Bass/Tile.

Problem (hardcoded shapes):
  x        [4, 2048, 1024] f32
  gate_w   [1024, 8] f32, gate_b [8] f32
  expert_w [8, 1024, 1024] f32, expert_b [8, 1024] f32
  out[b,t,p] = sum_e softmax(x @ gate_w + gate_b)[b,t,e]
               * (x @ expert_w[e] + expert_b[e])[b,t,p]

Sharding: data-parallel over tokens. 8192 tokens are split into 8 shards of
1024; every core gets the full gate/expert weights (replicated) and computes
its token shard end-to-end. No collectives.

Per-core kernel (x pre-transposed on host so the contraction dim is the
partition dim for both matmul operands):
  - gating logits per token tile via PE matmuls accumulated over d-tiles
    (gate_b broadcast in via a K=1 ones x gate_b rank-1 matmul), softmax on
    DVE/ACT, normalized gates also transposed on PE for the bias-mix matmul
  - expert e: psum[t128, p512] accumulates sum_d xT[d,t].T @ w_e[d,p] over
    8 d-tiles; d is the outer loop within a 4-token-tile half so compute
    starts as soon as the first w d-tile DMA lands
  - gate-weighted sum on DVE: acc = psum_e * g[:,e] + acc (one fused
    scalar_tensor_tensor per psum tile)
  - expert_b handled once per output tile: psum_b = gT.T @ expert_b (K=8
    matmul, gate-weighted bias mix), final out = acc + psum_b
Matmul dtype: bf16 (default) or float32r (full-rate fp32 streaming, ~1.2x
slower, ~16x more accurate) via MOE_MM_DTYPE in {bf16, fp32r, fp32}.
"""

import os
from contextlib import ExitStack

import numpy as np

import concourse.bacc as bacc
import concourse.bass as bass
import concourse.mybir as mybir
import concourse.tile as tile
from concourse.bass_utils import run_bass_kernel_spmd

B, T, D, E, P = 4, 2048, 1024, 8, 1024
N_CORES = 8
TOK = B * T                # 8192 tokens
TS = TOK // N_CORES        # 1024 tokens per core
DT = D // 128              # 8 contraction tiles
TT = TS // 128             # 8 token tiles per core
PCHUNK = 512               # psum bank free size (f32)
PC = P // PCHUNK           # 2 p-chunks
TH = 4                     # token tiles per half (TH*PC = 8 psum banks)

_F32 = mybir.dt.float32
_BF16 = mybir.dt.bfloat16

MM_DTYPE = os.environ.get("MOE_MM_DTYPE", "bf16")
TRACE = os.environ.get("MOE_TRACE", "0") == "1"

_mm_dt = {
    "fp32r": mybir.dt.float32r,
    "bf16": mybir.dt.bfloat16,
    "fp32": mybir.dt.float32,
}

_build_cache = {}


def _build(mode: str) -> bass.Bass:
    mm = _mm_dt[mode]
    nc = bacc.Bacc("TRN2", target_bir_lowering=False, debug=False,
                   num_devices=N_CORES)

    xT = nc.dram_tensor("xT", [D, TS], mm, kind="ExternalInput").ap()
    gw = nc.dram_tensor("gate_w", [D, E], mm, kind="ExternalInput").ap()
    gb = nc.dram_tensor("gate_b", [1, E], mm, kind="ExternalInput").ap()
    ew = nc.dram_tensor("expert_w", [E, D, P], mm, kind="ExternalInput").ap()
    eb = nc.dram_tensor("expert_b", [E, P], _BF16, kind="ExternalInput").ap()
    ones = nc.dram_tensor("ones", [1, 128], mm, kind="ExternalInput").ap()
    ident = nc.dram_tensor("ident", [128, 128], _F32, kind="ExternalInput").ap()
    out = nc.dram_tensor("out", [TS, P], _F32, kind="ExternalOutput").ap()

    out_t = out.rearrange("(tt tp) p -> tp tt p", tp=128)
    xT_t = xT.rearrange("(dt dp) t -> dp dt t", dp=128)

    with tile.TileContext(nc) as tc, ExitStack() as ctx:
        consts = ctx.enter_context(tc.tile_pool(name="consts", bufs=1))
        w_pool = ctx.enter_context(tc.tile_pool(name="w", bufs=22))
        stage_pool = ctx.enter_context(tc.tile_pool(name="stage", bufs=4))
        stats = ctx.enter_context(tc.tile_pool(name="stats", bufs=4))
        psum = ctx.enter_context(tc.tile_pool(name="psum", bufs=8, space="PSUM"))

        # Small resident inputs first, then xT and expert-0 weights
        # interleaved per d-tile so the expert-0 pipeline fills ASAP.
        ones_sb = consts.tile([1, 128], mm, name="ones_sb")
        nc.sync.dma_start(ones_sb[:, :], ones)
        gb_sb = consts.tile([1, E], mm, name="gb_sb")
        nc.sync.dma_start(gb_sb[:, :], gb)
        gw_sb = consts.tile([128, DT, E], mm, name="gw_sb")
        nc.sync.dma_start(gw_sb[:, :, :], gw.rearrange("(dt dp) e -> dp dt e", dp=128))
        eb_sb = consts.tile([E, P], _BF16, name="eb_sb")
        nc.sync.dma_start(eb_sb[:, :], eb)
        id_sb = consts.tile([128, 128], _F32, name="id_sb")
        nc.sync.dma_start(id_sb[:, :], ident)

        xt = consts.tile([128, DT, TS], mm, name="xt")
        w0 = []
        for di in range(DT):
            nc.sync.dma_start(xt[:, di, :], xT_t[:, di, :])
            w_tile = w_pool.tile([128, P], mm, name=f"wt0_{di}", tag="wt")
            nc.sync.dma_start(w_tile[:, :], ew[0, di * 128:(di + 1) * 128, :])
            w0.append(w_tile)

        g_sb = consts.tile([128, TT, E], _F32, name="g_sb")
        gt_sb = consts.tile([E, TS], _BF16, name="gt_sb")
        acc = consts.tile([128, TT, P], _F32, name="acc")

        # --- gating: g = softmax(x @ gate_w + gate_b), plus gT for the
        # bias-mix matmul ---
        for ti in range(TT):
            ps_g = psum.tile([128, PCHUNK], _F32, name="ps_g", tag="ps")
            lg = ps_g[:, :E]
            nc.tensor.matmul(lg, ones_sb[:1, :], gb_sb[:1, :],
                             start=True, stop=False)
            for di in range(DT):
                nc.tensor.matmul(lg, xt[:, di, ti * 128:(ti + 1) * 128],
                                 gw_sb[:, di, :],
                                 start=False, stop=(di == DT - 1))
            negmax = stats.tile([128, 1], _F32, name="negmax")
            nc.vector.tensor_reduce(negmax[:, :], lg, axis=mybir.AxisListType.X,
                                    op=mybir.AluOpType.max, negate=True)
            gexp = g_sb[:, ti, :]
            esum = stats.tile([128, 1], _F32, name="esum")
            nc.scalar.activation(gexp, lg, mybir.ActivationFunctionType.Exp,
                                 bias=negmax[:, :], scale=1.0,
                                 accum_out=esum[:, :])
            rec = stats.tile([128, 1], _F32, name="rec")
            nc.vector.reciprocal(rec[:, :], esum[:, :])
            nc.vector.tensor_scalar_mul(gexp, gexp, rec[:, :])
            # gT[e, t] for the expert_b bias-mix matmul
            ps_t = psum.tile([128, PCHUNK], _F32, name="ps_t", tag="ps")
            gt_ps = ps_t[:E, :128]
            nc.tensor.transpose(gt_ps, gexp, id_sb[:, :])
            nc.scalar.copy(gt_sb[:, ti * 128:(ti + 1) * 128], gt_ps)

        # --- experts ---
        def epilogue(e, ti, pc, ps):
            g_col = g_sb[:, ti, e:e + 1]
            acc_sl = acc[:, ti, pc * PCHUNK:(pc + 1) * PCHUNK]
            if e == 0:
                nc.vector.tensor_scalar_mul(acc_sl, ps[:, :], g_col)
            else:
                nc.vector.scalar_tensor_tensor(
                    acc_sl, ps[:, :], g_col, acc_sl,
                    op0=mybir.AluOpType.mult, op1=mybir.AluOpType.add)
            if e == E - 1:
                # gate-weighted expert_b mix + final store
                ps_b = psum.tile([128, PCHUNK], _F32,
                                 name=f"psb{ti}_{pc}", tag="ps")
                nc.tensor.matmul(
                    ps_b[:, :], gt_sb[:, ti * 128:(ti + 1) * 128],
                    eb_sb[:, pc * PCHUNK:(pc + 1) * PCHUNK],
                    start=True, stop=True)
                stg = stage_pool.tile([128, PCHUNK], _F32, name="stg")
                nc.vector.tensor_add(stg[:, :], acc_sl, ps_b[:, :])
                nc.sync.dma_start(
                    out_t[:, ti, pc * PCHUNK:(pc + 1) * PCHUNK], stg[:, :])

        for e in range(E):
            if e == 0:
                wt = w0
            else:
                wt = []
                for di in range(DT):
                    w_tile = w_pool.tile([128, P], mm, name=f"wt{e}_{di}",
                                         tag="wt")
                    nc.sync.dma_start(w_tile[:, :],
                                      ew[e, di * 128:(di + 1) * 128, :])
                    wt.append(w_tile)
            if e == 0:
                # d-outer: start computing as soon as the first xT/w0
                # d-tiles land (DMA-bound ramp-in phase).
                for half in range(TT // TH):
                    tis = range(half * TH, (half + 1) * TH)
                    ps_grp = {}
                    for ti in tis:
                        for pc in range(PC):
                            ps_grp[ti, pc] = psum.tile(
                                [128, PCHUNK], _F32,
                                name=f"ps{e}_{ti}_{pc}", tag="ps")
                    for di in range(DT):
                        for ti in tis:
                            for pc in range(PC):
                                nc.tensor.matmul(
                                    ps_grp[ti, pc][:, :],
                                    xt[:, di, ti * 128:(ti + 1) * 128],
                                    wt[di][:, pc * PCHUNK:(pc + 1) * PCHUNK],
                                    start=(di == 0), stop=(di == DT - 1))
                    for ti in tis:
                        for pc in range(PC):
                            epilogue(e, ti, pc, ps_grp[ti, pc])
            else:
                # group-major: each output tile finishes its d-loop early so
                # the DVE epilogue chain spreads across the expert phase.
                for ti in range(TT):
                    for pc in range(PC):
                        ps = psum.tile([128, PCHUNK], _F32,
                                       name=f"ps{e}_{ti}_{pc}", tag="ps")
                        for di in range(DT):
                            nc.tensor.matmul(
                                ps[:, :], xt[:, di, ti * 128:(ti + 1) * 128],
                                wt[di][:, pc * PCHUNK:(pc + 1) * PCHUNK],
                                start=(di == 0), stop=(di == DT - 1))
                        epilogue(e, ti, pc, ps)

    nc.compile()
    return nc


def _get_module(mode: str) -> bass.Bass:
    if mode not in _build_cache:
        _build_cache[mode] = _build(mode)
    return _build_cache[mode]


_last_results = None


def _host_inputs(x, gate_w, gate_b, expert_w, expert_b, mode):
    import ml_dtypes
    np_dt = ml_dtypes.bfloat16 if mode == "bf16" else np.float32

    x_flat = np.asarray(x, dtype=np.float32).reshape(TOK, D)
    gw_h = np.ascontiguousarray(np.asarray(gate_w, np.float32)).astype(np_dt)
    gb_h = np.asarray(gate_b, np.float32).reshape(1, E).astype(np_dt)
    ew_h = np.ascontiguousarray(np.asarray(expert_w, np.float32)).astype(np_dt)
    eb_h = np.asarray(expert_b, np.float32).astype(ml_dtypes.bfloat16)
    ones_h = np.ones((1, 128), dtype=np_dt)
    ident_h = np.eye(128, dtype=np.float32)

    in_maps = []
    for c in range(N_CORES):
        shard = x_flat[c * TS:(c + 1) * TS]                  # [TS, D]
        xT_h = np.ascontiguousarray(shard.T).astype(np_dt)   # [D, TS]
        in_maps.append({
            "xT": xT_h, "gate_w": gw_h, "gate_b": gb_h,
            "expert_w": ew_h, "expert_b": eb_h, "ones": ones_h,
            "ident": ident_h,
        })
    return in_maps


def kernel(x, gate_w, gate_b, expert_w, expert_b):
    global _last_results
    mode = MM_DTYPE
    nc = _get_module(mode)
    in_maps = _host_inputs(x, gate_w, gate_b, expert_w, expert_b, mode)

    res = run_bass_kernel_spmd(nc, in_maps, core_ids=list(range(N_CORES)),
                               trace=TRACE)
    _last_results = res

    out = np.concatenate([res.results[c]["out"] for c in range(N_CORES)], axis=0)
    return out.reshape(B, T, P).astype(np.float32)


# revision 11
# speedup vs baseline: 1.0773x; 1.0507x over previous
"""MoE layer (dense experts) on 8 Trainium2 NeuronCores via Bass/Tile.

Problem (hardcoded shapes):
  x        [4, 2048, 1024] f32
  gate_w   [1024, 8] f32, gate_b [8] f32
  expert_w [8, 1024, 1024] f32, expert_b [8, 1024] f32
  out[b,t,p] = sum_e softmax(x @ gate_w + gate_b)[b,t,e]
               * (x @ expert_w[e] + expert_b[e])[b,t,p]

Sharding: data-parallel over tokens. 8192 tokens are split into 8 shards of
1024; every core gets the full gate/expert weights (replicated) and computes
its token shard end-to-end. No collectives.

Per-core kernel (x pre-transposed on host so the contraction dim is the
partition dim for both matmul operands):
  - gating logits per token tile via PE matmuls accumulated over d-tiles
    (gate_b broadcast in via a K=1 ones x gate_b rank-1 matmul), softmax on
    DVE/ACT, normalized gates also transposed on PE for the bias-mix matmul
  - expert e: psum[t128, p512] accumulates sum_d xT[d,t].T @ w_e[d,p] over
    8 d-tiles; d is the outer loop within a 4-token-tile half so compute
    starts as soon as the first w d-tile DMA lands
  - gate-weighted sum on DVE: acc = psum_e * g[:,e] + acc (one fused
    scalar_tensor_tensor per psum tile)
  - expert_b handled once per output tile: psum_b = gT.T @ expert_b (K=8
    matmul, gate-weighted bias mix), final out = acc + psum_b
Matmul dtype: bf16 (default) or float32r (full-rate fp32 streaming, ~1.2x
slower, ~16x more accurate) via MOE_MM_DTYPE in {bf16, fp32r, fp32}.
"""

import os
from contextlib import ExitStack

import numpy as np

import concourse.bacc as bacc
import concourse.bass as bass
import concourse.mybir as mybir
import concourse.tile as tile
from concourse.bass_utils import run_bass_kernel_spmd

B, T, D, E, P = 4, 2048, 1024, 8, 1024
N_CORES = 8
TOK = B * T                # 8192 tokens
TS = TOK // N_CORES        # 1024 tokens per core
DT = D // 128              # 8 contraction tiles
TT = TS // 128             # 8 token tiles per core
PCHUNK = 512               # psum bank free size (f32)
PC = P // PCHUNK           # 2 p-chunks
TH = 4                     # token tiles per half (TH*PC = 8 psum banks)

_F32 = mybir.dt.float32
_BF16 = mybir.dt.bfloat16

MM_DTYPE = os.environ.get("MOE_MM_DTYPE", "bf16")
TRACE = os.environ.get("MOE_TRACE", "0") == "1"

_mm_dt = {
    "fp32r": mybir.dt.float32r,
    "bf16": mybir.dt.bfloat16,
    "fp32": mybir.dt.float32,
}

_build_cache = {}


def _build(mode: str) -> bass.Bass:
    mm = _mm_dt[mode]
    nc = bacc.Bacc("TRN2", target_bir_lowering=False, debug=False,
                   num_devices=N_CORES)

    xT = nc.dram_tensor("xT", [D, TS], mm, kind="ExternalInput").ap()
    gw = nc.dram_tensor("gate_w", [D, E], mm, kind="ExternalInput").ap()
    gb = nc.dram_tensor("gate_b", [1, E], mm, kind="ExternalInput").ap()
    ew = nc.dram_tensor("expert_w", [E, D, P], mm, kind="ExternalInput").ap()
    eb = nc.dram_tensor("expert_b", [E, P], _BF16, kind="ExternalInput").ap()
    ones = nc.dram_tensor("ones", [1, 128], mm, kind="ExternalInput").ap()
    ident = nc.dram_tensor("ident", [128, 128], _F32, kind="ExternalInput").ap()
    out = nc.dram_tensor("out", [TS, P], _F32, kind="ExternalOutput").ap()

    out_t = out.rearrange("(tt tp) p -> tp tt p", tp=128)
    xT_t = xT.rearrange("(dt dp) t -> dp dt t", dp=128)

    with tile.TileContext(nc) as tc, ExitStack() as ctx:
        consts = ctx.enter_context(tc.tile_pool(name="consts", bufs=1))
        w_pool = ctx.enter_context(tc.tile_pool(name="w", bufs=22))
        stage_pool = ctx.enter_context(tc.tile_pool(name="stage", bufs=4))
        stats = ctx.enter_context(tc.tile_pool(name="stats", bufs=4))
        psum = ctx.enter_context(tc.tile_pool(name="psum", bufs=8, space="PSUM"))

        # Small resident inputs first, then xT and expert-0 weights
        # interleaved per d-tile so the expert-0 pipeline fills ASAP.
        ones_sb = consts.tile([1, 128], mm, name="ones_sb")
        nc.sync.dma_start(ones_sb[:, :], ones)
        gb_sb = consts.tile([1, E], mm, name="gb_sb")
        nc.sync.dma_start(gb_sb[:, :], gb)
        gw_sb = consts.tile([128, DT, E], mm, name="gw_sb")
        nc.sync.dma_start(gw_sb[:, :, :], gw.rearrange("(dt dp) e -> dp dt e", dp=128))
        eb_sb = consts.tile([E, P], _BF16, name="eb_sb")
        nc.sync.dma_start(eb_sb[:, :], eb)
        id_sb = consts.tile([128, 128], _F32, name="id_sb")
        nc.sync.dma_start(id_sb[:, :], ident)

        xt = consts.tile([128, DT, TS], mm, name="xt")
        for di in range(DT):
            nc.sync.dma_start(xt[:, di, :], xT_t[:, di, :])
        w0 = []
        for di in range(DT):
            w_tile = w_pool.tile([128, P], mm, name=f"wt0_{di}", tag="wt")
            nc.sync.dma_start(w_tile[:, :], ew[0, di * 128:(di + 1) * 128, :])
            w0.append(w_tile)

        g_sb = consts.tile([128, TT, E], _F32, name="g_sb")
        gt_sb = consts.tile([E, TS], _BF16, name="gt_sb")
        acc = consts.tile([128, TT, P], _F32, name="acc")

        # --- gating: g = softmax(x @ gate_w + gate_b), plus gT for the
        # bias-mix matmul. Logits accumulate d-outer into two packed psum
        # banks (4 token tiles x 8 logits each) so gating starts as soon as
        # the first xT d-tile lands instead of waiting for all of xT.
        lg_bank = [psum.tile([128, TH * E], _F32, name=f"lg{h}", tag="ps")
                   for h in range(TT // TH)]

        def lg_slice(ti):
            return lg_bank[ti // TH][:, (ti % TH) * E:(ti % TH + 1) * E]

        for ti in range(TT):
            nc.tensor.matmul(lg_slice(ti), ones_sb[:1, :], gb_sb[:1, :],
                             start=True, stop=False)
        for di in range(DT):
            for ti in range(TT):
                nc.tensor.matmul(lg_slice(ti),
                                 xt[:, di, ti * 128:(ti + 1) * 128],
                                 gw_sb[:, di, :],
                                 start=False, stop=(di == DT - 1))
        for ti in range(TT):
            lg = lg_slice(ti)
            negmax = stats.tile([128, 1], _F32, name="negmax")
            nc.vector.tensor_reduce(negmax[:, :], lg, axis=mybir.AxisListType.X,
                                    op=mybir.AluOpType.max, negate=True)
            gexp = g_sb[:, ti, :]
            esum = stats.tile([128, 1], _F32, name="esum")
            nc.scalar.activation(gexp, lg, mybir.ActivationFunctionType.Exp,
                                 bias=negmax[:, :], scale=1.0,
                                 accum_out=esum[:, :])
            rec = stats.tile([128, 1], _F32, name="rec")
            nc.vector.reciprocal(rec[:, :], esum[:, :])
            nc.vector.tensor_scalar_mul(gexp, gexp, rec[:, :])
            # gT[e, t] for the expert_b bias-mix matmul
            ps_t = psum.tile([128, PCHUNK], _F32, name="ps_t", tag="ps")
            gt_ps = ps_t[:E, :128]
            nc.tensor.transpose(gt_ps, gexp, id_sb[:, :])
            nc.scalar.copy(gt_sb[:, ti * 128:(ti + 1) * 128], gt_ps)

        # --- experts ---
        def epilogue(e, ti, pc, ps):
            g_col = g_sb[:, ti, e:e + 1]
            acc_sl = acc[:, ti, pc * PCHUNK:(pc + 1) * PCHUNK]
            if e == 0:
                nc.vector.tensor_scalar_mul(acc_sl, ps[:, :], g_col)
            else:
                nc.vector.scalar_tensor_tensor(
                    acc_sl, ps[:, :], g_col, acc_sl,
                    op0=mybir.AluOpType.mult, op1=mybir.AluOpType.add)
            if e == E - 1:
                # gate-weighted expert_b mix + final store
                ps_b = psum.tile([128, PCHUNK], _F32,
                                 name=f"psb{ti}_{pc}", tag="ps")
                nc.tensor.matmul(
                    ps_b[:, :], gt_sb[:, ti * 128:(ti + 1) * 128],
                    eb_sb[:, pc * PCHUNK:(pc + 1) * PCHUNK],
                    start=True, stop=True)
                stg = stage_pool.tile([128, PCHUNK], _F32, name="stg")
                nc.vector.tensor_add(stg[:, :], acc_sl, ps_b[:, :])
                nc.sync.dma_start(
                    out_t[:, ti, pc * PCHUNK:(pc + 1) * PCHUNK], stg[:, :])

        for e in range(E):
            if e == 0:
                wt = w0
            else:
                wt = []
                for di in range(DT):
                    w_tile = w_pool.tile([128, P], mm, name=f"wt{e}_{di}",
                                         tag="wt")
                    nc.sync.dma_start(w_tile[:, :],
                                      ew[e, di * 128:(di + 1) * 128, :])
                    wt.append(w_tile)
            if e == 0:
                # First half d-outer: start computing as soon as the first
                # w0 d-tiles land (DMA-bound ramp-in phase). Second half
                # group-major so its epilogue chains spread out.
                tis = range(TH)
                ps_grp = {}
                for ti in tis:
                    for pc in range(PC):
                        ps_grp[ti, pc] = psum.tile(
                            [128, PCHUNK], _F32,
                            name=f"ps{e}_{ti}_{pc}", tag="ps")
                for di in range(DT):
                    for ti in tis:
                        for pc in range(PC):
                            nc.tensor.matmul(
                                ps_grp[ti, pc][:, :],
                                xt[:, di, ti * 128:(ti + 1) * 128],
                                wt[di][:, pc * PCHUNK:(pc + 1) * PCHUNK],
                                start=(di == 0), stop=(di == DT - 1))
                for ti in tis:
                    for pc in range(PC):
                        epilogue(e, ti, pc, ps_grp[ti, pc])
                for ti in range(TH, TT):
                    for pc in range(PC):
                        ps = psum.tile([128, PCHUNK], _F32,
                                       name=f"ps{e}_{ti}_{pc}", tag="ps")
                        for di in range(DT):
                            nc.tensor.matmul(
                                ps[:, :], xt[:, di, ti * 128:(ti + 1) * 128],
                                wt[di][:, pc * PCHUNK:(pc + 1) * PCHUNK],
                                start=(di == 0), stop=(di == DT - 1))
                        epilogue(e, ti, pc, ps)
            else:
                # group-major: each output tile finishes its d-loop early so
                # the DVE epilogue chain spreads across the expert phase.
                for ti in range(TT):
                    for pc in range(PC):
                        ps = psum.tile([128, PCHUNK], _F32,
                                       name=f"ps{e}_{ti}_{pc}", tag="ps")
                        for di in range(DT):
                            nc.tensor.matmul(
                                ps[:, :], xt[:, di, ti * 128:(ti + 1) * 128],
                                wt[di][:, pc * PCHUNK:(pc + 1) * PCHUNK],
                                start=(di == 0), stop=(di == DT - 1))
                        epilogue(e, ti, pc, ps)

    nc.compile()
    return nc


def _get_module(mode: str) -> bass.Bass:
    if mode not in _build_cache:
        _build_cache[mode] = _build(mode)
    return _build_cache[mode]


_last_results = None


def _host_inputs(x, gate_w, gate_b, expert_w, expert_b, mode):
    import ml_dtypes
    np_dt = ml_dtypes.bfloat16 if mode == "bf16" else np.float32

    x_flat = np.asarray(x, dtype=np.float32).reshape(TOK, D)
    gw_h = np.ascontiguousarray(np.asarray(gate_w, np.float32)).astype(np_dt)
    gb_h = np.asarray(gate_b, np.float32).reshape(1, E).astype(np_dt)
    ew_h = np.ascontiguousarray(np.asarray(expert_w, np.float32)).astype(np_dt)
    eb_h = np.asarray(expert_b, np.float32).astype(ml_dtypes.bfloat16)
    ones_h = np.ones((1, 128), dtype=np_dt)
    ident_h = np.eye(128, dtype=np.float32)

    in_maps = []
    for c in range(N_CORES):
        shard = x_flat[c * TS:(c + 1) * TS]                  # [TS, D]
        xT_h = np.ascontiguousarray(shard.T).astype(np_dt)   # [D, TS]
        in_maps.append({
            "xT": xT_h, "gate_w": gw_h, "gate_b": gb_h,
            "expert_w": ew_h, "expert_b": eb_h, "ones": ones_h,
            "ident": ident_h,
        })
    return in_maps


def kernel(x, gate_w, gate_b, expert_w, expert_b):
    global _last_results
    mode = MM_DTYPE
    nc = _get_module(mode)
    in_maps = _host_inputs(x, gate_w, gate_b, expert_w, expert_b, mode)

    res = run_bass_kernel_spmd(nc, in_maps, core_ids=list(range(N_CORES)),
                               trace=TRACE)
    _last_results = res

    out = np.concatenate([res.results[c]["out"] for c in range(N_CORES)], axis=0)
    return out.reshape(B, T, P).astype(np.float32)


# revision 12
# speedup vs baseline: 1.0827x; 1.0050x over previous
"""MoE layer (dense experts) on 8 Trainium2 NeuronCores via Bass/Tile.

Problem (hardcoded shapes):
  x        [4, 2048, 1024] f32
  gate_w   [1024, 8] f32, gate_b [8] f32
  expert_w [8, 1024, 1024] f32, expert_b [8, 1024] f32
  out[b,t,p] = sum_e softmax(x @ gate_w + gate_b)[b,t,e]
               * (x @ expert_w[e] + expert_b[e])[b,t,p]

Sharding: data-parallel over tokens. 8192 tokens are split into 8 shards of
1024; every core gets the full gate/expert weights (replicated) and computes
its token shard end-to-end. No collectives.

Per-core kernel (x pre-transposed on host so the contraction dim is the
partition dim for both matmul operands):
  - gating logits per token tile via PE matmuls accumulated over d-tiles
    (gate_b broadcast in via a K=1 ones x gate_b rank-1 matmul), softmax on
    DVE/ACT, normalized gates also transposed on PE for the bias-mix matmul
  - expert e: psum[t128, p512] accumulates sum_d xT[d,t].T @ w_e[d,p] over
    8 d-tiles; d is the outer loop within a 4-token-tile half so compute
    starts as soon as the first w d-tile DMA lands
  - gate-weighted sum on DVE: acc = psum_e * g[:,e] + acc (one fused
    scalar_tensor_tensor per psum tile)
  - expert_b handled once per output tile: psum_b = gT.T @ expert_b (K=8
    matmul, gate-weighted bias mix), final out = acc + psum_b
Matmul dtype: bf16 (default) or float32r (full-rate fp32 streaming, ~1.2x
slower, ~16x more accurate) via MOE_MM_DTYPE in {bf16, fp32r, fp32}.
"""

import os
from contextlib import ExitStack

import numpy as np

import concourse.bacc as bacc
import concourse.bass as bass
import concourse.mybir as mybir
import concourse.tile as tile
from concourse.bass_utils import run_bass_kernel_spmd

B, T, D, E, P = 4, 2048, 1024, 8, 1024
N_CORES = 8
TOK = B * T                # 8192 tokens
TS = TOK // N_CORES        # 1024 tokens per core
DT = D // 128              # 8 contraction tiles
TT = TS // 128             # 8 token tiles per core
PCHUNK = 512               # psum bank free size (f32)
PC = P // PCHUNK           # 2 p-chunks
TH = 4                     # token tiles per half (TH*PC = 8 psum banks)

_F32 = mybir.dt.float32
_BF16 = mybir.dt.bfloat16

MM_DTYPE = os.environ.get("MOE_MM_DTYPE", "bf16")
TRACE = os.environ.get("MOE_TRACE", "0") == "1"

_mm_dt = {
    "fp32r": mybir.dt.float32r,
    "bf16": mybir.dt.bfloat16,
    "fp32": mybir.dt.float32,
}

_build_cache = {}


def _build(mode: str) -> bass.Bass:
    mm = _mm_dt[mode]
    nc = bacc.Bacc("TRN2", target_bir_lowering=False, debug=False,
                   num_devices=N_CORES)

    xT = nc.dram_tensor("xT", [D, TS], mm, kind="ExternalInput").ap()
    gw = nc.dram_tensor("gate_w", [D, E], mm, kind="ExternalInput").ap()
    gb = nc.dram_tensor("gate_b", [1, E], mm, kind="ExternalInput").ap()
    ew = nc.dram_tensor("expert_w", [E, D, P], mm, kind="ExternalInput").ap()
    eb = nc.dram_tensor("expert_b", [E, P], _BF16, kind="ExternalInput").ap()
    ones = nc.dram_tensor("ones", [1, 128], mm, kind="ExternalInput").ap()
    ident = nc.dram_tensor("ident", [128, 128], _F32, kind="ExternalInput").ap()
    out = nc.dram_tensor("out", [TS, P], _F32, kind="ExternalOutput").ap()

    out_t = out.rearrange("(tt tp) p -> tp tt p", tp=128)
    xT_t = xT.rearrange("(dt dp) t -> dp dt t", dp=128)

    with tile.TileContext(nc) as tc, ExitStack() as ctx:
        consts = ctx.enter_context(tc.tile_pool(name="consts", bufs=1))
        w_pool = ctx.enter_context(tc.tile_pool(name="w", bufs=22))
        stage_pool = ctx.enter_context(tc.tile_pool(name="stage", bufs=4))
        stats = ctx.enter_context(tc.tile_pool(name="stats", bufs=4))
        psum = ctx.enter_context(tc.tile_pool(name="psum", bufs=8, space="PSUM"))

        # Small resident inputs first, then xT and expert-0 weights
        # interleaved per d-tile so the expert-0 pipeline fills ASAP.
        ones_sb = consts.tile([1, 128], mm, name="ones_sb")
        nc.sync.dma_start(ones_sb[:, :], ones)
        gb_sb = consts.tile([1, E], mm, name="gb_sb")
        nc.sync.dma_start(gb_sb[:, :], gb)
        gw_sb = consts.tile([128, DT, E], mm, name="gw_sb")
        nc.sync.dma_start(gw_sb[:, :, :], gw.rearrange("(dt dp) e -> dp dt e", dp=128))
        eb_sb = consts.tile([E, P], _BF16, name="eb_sb")
        nc.sync.dma_start(eb_sb[:, :], eb)
        id_sb = consts.tile([128, 128], _F32, name="id_sb")
        nc.sync.dma_start(id_sb[:, :], ident)

        xt = consts.tile([128, DT, TS], mm, name="xt")
        for di in range(DT):
            nc.sync.dma_start(xt[:, di, :], xT_t[:, di, :])
        w0 = []
        for di in range(DT):
            w_tile = w_pool.tile([128, P], mm, name=f"wt0_{di}", tag="wt")
            nc.sync.dma_start(w_tile[:, :], ew[0, di * 128:(di + 1) * 128, :])
            w0.append(w_tile)

        g_sb = consts.tile([128, TT, E], _F32, name="g_sb")
        gt_sb = consts.tile([E, TS], _BF16, name="gt_sb")
        acc = consts.tile([128, TT, P], _F32, name="acc")

        # --- gating: g = softmax(x @ gate_w + gate_b), plus gT for the
        # bias-mix matmul. Logits accumulate d-outer (one psum bank per
        # token tile) so gating starts as soon as the first xT d-tile lands
        # instead of waiting for all of xT.
        lg_bank = [psum.tile([128, E], _F32, name=f"lg{ti}", tag="ps")
                   for ti in range(TT)]
        for ti in range(TT):
            nc.tensor.matmul(lg_bank[ti][:, :], ones_sb[:1, :], gb_sb[:1, :],
                             start=True, stop=False)
        for di in range(DT):
            for ti in range(TT):
                nc.tensor.matmul(lg_bank[ti][:, :],
                                 xt[:, di, ti * 128:(ti + 1) * 128],
                                 gw_sb[:, di, :],
                                 start=False, stop=(di == DT - 1))
        for ti in range(TT):
            lg = lg_bank[ti][:, :]
            negmax = stats.tile([128, 1], _F32, name="negmax")
            nc.vector.tensor_reduce(negmax[:, :], lg, axis=mybir.AxisListType.X,
                                    op=mybir.AluOpType.max, negate=True)
            gexp = g_sb[:, ti, :]
            esum = stats.tile([128, 1], _F32, name="esum")
            nc.scalar.activation(gexp, lg, mybir.ActivationFunctionType.Exp,
                                 bias=negmax[:, :], scale=1.0,
                                 accum_out=esum[:, :])
            rec = stats.tile([128, 1], _F32, name="rec")
            nc.vector.reciprocal(rec[:, :], esum[:, :])
            nc.vector.tensor_scalar_mul(gexp, gexp, rec[:, :])
            # gT[e, t] for the expert_b bias-mix matmul
            ps_t = psum.tile([128, PCHUNK], _F32, name="ps_t", tag="ps")
            gt_ps = ps_t[:E, :128]
            nc.tensor.transpose(gt_ps, gexp, id_sb[:, :])
            nc.scalar.copy(gt_sb[:, ti * 128:(ti + 1) * 128], gt_ps)

        # --- experts ---
        def epilogue(e, ti, pc, ps):
            g_col = g_sb[:, ti, e:e + 1]
            acc_sl = acc[:, ti, pc * PCHUNK:(pc + 1) * PCHUNK]
            if e == 0:
                nc.vector.tensor_scalar_mul(acc_sl, ps[:, :], g_col)
            else:
                nc.vector.scalar_tensor_tensor(
                    acc_sl, ps[:, :], g_col, acc_sl,
                    op0=mybir.AluOpType.mult, op1=mybir.AluOpType.add)
            if e == E - 1:
                # gate-weighted expert_b mix + final store
                ps_b = psum.tile([128, PCHUNK], _F32,
                                 name=f"psb{ti}_{pc}", tag="ps")
                nc.tensor.matmul(
                    ps_b[:, :], gt_sb[:, ti * 128:(ti + 1) * 128],
                    eb_sb[:, pc * PCHUNK:(pc + 1) * PCHUNK],
                    start=True, stop=True)
                stg = stage_pool.tile([128, PCHUNK], _F32, name="stg")
                nc.vector.tensor_add(stg[:, :], acc_sl, ps_b[:, :])
                nc.sync.dma_start(
                    out_t[:, ti, pc * PCHUNK:(pc + 1) * PCHUNK], stg[:, :])

        for e in range(E):
            if e == 0:
                wt = w0
            else:
                wt = []
                for di in range(DT):
                    w_tile = w_pool.tile([128, P], mm, name=f"wt{e}_{di}",
                                         tag="wt")
                    nc.sync.dma_start(w_tile[:, :],
                                      ew[e, di * 128:(di + 1) * 128, :])
                    wt.append(w_tile)
            if e == 0:
                # First half d-outer: start computing as soon as the first
                # w0 d-tiles land (DMA-bound ramp-in phase). Second half
                # group-major so its epilogue chains spread out.
                tis = range(TH)
                ps_grp = {}
                for ti in tis:
                    for pc in range(PC):
                        ps_grp[ti, pc] = psum.tile(
                            [128, PCHUNK], _F32,
                            name=f"ps{e}_{ti}_{pc}", tag="ps")
                for di in range(DT):
                    for ti in tis:
                        for pc in range(PC):
                            nc.tensor.matmul(
                                ps_grp[ti, pc][:, :],
                                xt[:, di, ti * 128:(ti + 1) * 128],
                                wt[di][:, pc * PCHUNK:(pc + 1) * PCHUNK],
                                start=(di == 0), stop=(di == DT - 1))
                for ti in tis:
                    for pc in range(PC):
                        epilogue(e, ti, pc, ps_grp[ti, pc])
                for ti in range(TH, TT):
                    for pc in range(PC):
                        ps = psum.tile([128, PCHUNK], _F32,
                                       name=f"ps{e}_{ti}_{pc}", tag="ps")
                        for di in range(DT):
                            nc.tensor.matmul(
                                ps[:, :], xt[:, di, ti * 128:(ti + 1) * 128],
                                wt[di][:, pc * PCHUNK:(pc + 1) * PCHUNK],
                                start=(di == 0), stop=(di == DT - 1))
                        epilogue(e, ti, pc, ps)
            else:
                # group-major: each output tile finishes its d-loop early so
                # the DVE epilogue chain spreads across the expert phase.
                for ti in range(TT):
                    for pc in range(PC):
                        ps = psum.tile([128, PCHUNK], _F32,
                                       name=f"ps{e}_{ti}_{pc}", tag="ps")
                        for di in range(DT):
                            nc.tensor.matmul(
                                ps[:, :], xt[:, di, ti * 128:(ti + 1) * 128],
                                wt[di][:, pc * PCHUNK:(pc + 1) * PCHUNK],
                                start=(di == 0), stop=(di == DT - 1))
                        epilogue(e, ti, pc, ps)

    nc.compile()
    return nc


def _get_module(mode: str) -> bass.Bass:
    if mode not in _build_cache:
        _build_cache[mode] = _build(mode)
    return _build_cache[mode]


_last_results = None


def _host_inputs(x, gate_w, gate_b, expert_w, expert_b, mode):
    import ml_dtypes
    np_dt = ml_dtypes.bfloat16 if mode == "bf16" else np.float32

    x_flat = np.asarray(x, dtype=np.float32).reshape(TOK, D)
    gw_h = np.ascontiguousarray(np.asarray(gate_w, np.float32)).astype(np_dt)
    gb_h = np.asarray(gate_b, np.float32).reshape(1, E).astype(np_dt)
    ew_h = np.ascontiguousarray(np.asarray(expert_w, np.float32)).astype(np_dt)
    eb_h = np.asarray(expert_b, np.float32).astype(ml_dtypes.bfloat16)
    ones_h = np.ones((1, 128), dtype=np_dt)
    ident_h = np.eye(128, dtype=np.float32)

    in_maps = []
    for c in range(N_CORES):
        shard = x_flat[c * TS:(c + 1) * TS]                  # [TS, D]
        xT_h = np.ascontiguousarray(shard.T).astype(np_dt)   # [D, TS]
        in_maps.append({
            "xT": xT_h, "gate_w": gw_h, "gate_b": gb_h,
            "expert_w": ew_h, "expert_b": eb_h, "ones": ones_h,
            "ident": ident_h,
        })
    return in_maps


def kernel(x, gate_w, gate_b, expert_w, expert_b):
    global _last_results
    mode = MM_DTYPE
    nc = _get_module(mode)
    in_maps = _host_inputs(x, gate_w, gate_b, expert_w, expert_b, mode)

    res = run_bass_kernel_spmd(nc, in_maps, core_ids=list(range(N_CORES)),
                               trace=TRACE)
    _last_results = res

    out = np.concatenate([res.results[c]["out"] for c in range(N_CORES)], axis=0)
    return out.reshape(B, T, P).astype(np.float32)
